# revision 1
# baseline (speedup 1.0000x reference)
"""Trainium2 Bass kernel for nn_AttentionMechanism_21646635172225.

Reference computation (per batch element n):
    q   = transpose(x[n], (T,C,H,W)).reshape(T, C*H*W)      # x[n]: (C,T,H,W)
    E   = q @ q.T                                            # (T, T)
    A   = softmax(E, axis=-1)
    out = alpha * (A @ q) + q          -> reshape/transpose back to (C,T,H,W)

Sharding: data-parallel over batch N=8 across the 8 NeuronCores (one batch
element per core), alpha replicated.

Per-core dataflow (C=128 on partitions, free axis = t*784 + hw):
  Phase 1, pipelined over nslot hw-striped chunks:
    - DMA the chunk of x into SBUF (XNQ, native layout, 784B runs).
    - GpSimd casts it to bf16 into a rotating chunk slot (XNbf).
    - TensorE accumulates the energy Gram matrix with 4-hw-packed bf16
      matmuls (128-column weights -> FWL weight loads) into PSUM P4; the
      packing leaves 4 diagonal 32x32 blocks to sum later.
    - VectorE 32x32 block-transposes the chunk into the "folded t-major"
      layout qt[32g+t, cl*stride + jj] = q[t, 32g+cl, hw].  The transpose of
      slot m writes slot m-1's (dead) region of XNQ, slot 0 a spare tail
      region, so no second full-size buffer exists.
    - ScalarE pre-casts the folded chunk to bf16 (qtb) for the phase-2
      matmuls (slot 3's casts are emitted after softmax to keep the ScalarE
      queue clear for it).
  Softmax: diagonal blocks of P4 are summed and replicated to the 4
    partition groups with accumulating selector matmuls; softmax runs on all
    128 lanes (Exp's accum_out provides the row sums); alpha is folded in
    (B = alpha*attn [+ I]); a 32x32 block transpose gives B^T per group.
  Phase 2, per slot: TensorE computes alpha*attn @ q (bf16, 4 concurrent
    32x32 tiles via tile_position); VectorE adds the exact fp32 residual
    from PSUM onto qt ("exact" mode; alpha=0 stays bitwise exact since
    0-weight matmuls produce exact zeros); slot halves DMA to HBM (y kept
    in the folded layout, de-folded on host).
"""

import sys

sys.path.insert(0, "/opt/trn_rl_repo")

from contextlib import ExitStack

import numpy as np

import concourse.bass as bass
import concourse.tile as tile
from concourse import bacc, mybir

# Problem shape (hardcoded per contract)
N, C, T, H, W = 8, 128, 32, 28, 28
HB = H * W  # 784
F = T * HB  # 25088
G = 4  # partition groups (c blocks of 32)
CL = 32  # c-local within group
NCORES = 8

f32 = mybir.dt.float32
bf16 = mybir.dt.bfloat16
AF = mybir.ActivationFunctionType
ALU = mybir.AluOpType
AX = mybir.AxisListType


def build_nc(
    mode: str = "exact",  # "exact" | "fused"
    nslot: int = 4,  # hw-striped chunks/slots (4 | HB/nslot required)
    nmm: int = 392,  # matmul2 moving free size
    cast_sub: int = 7,  # cast pieces per chunk (Js/cast_sub must be mult of epack)
    gs_num: int = 0,  # of every gs_den TT groups, this many go via GpSimd
    gs_den: int = 2,
    stores_per_slot: int = 2,
    epack: int = 4,  # hw columns per energy matmul (1 or 4)
    cast_engine: str = "scalar",  # engine for x->bf16 casts
    qtb_ahead: bool = False,  # pre-cast folded q to bf16 during phase 1
    qtb_gp_slots: tuple = (),  # qtb slots cast by GpSimd during phase 1
    qtb_late: int = 2,  # this many trailing slots' qtb cast after softmax
    defer_last_t: bool = False,  # emit last slot's transpose after slot-0 TTs
    nspare: int = 1,  # spare compact qt regions (slots 0..nspare-1 contiguous)
    qtb_bufs: int = 0,  # qtb pool slots (0 = all groups resident)
    nstripe: int = 4,  # DMA/transpose granularity (nslot or 2*nslot)
):
    assert nstripe in (nslot, 2 * nslot)
    assert HB % nslot == 0
    Js = HB // nslot  # hw per chunk/slot
    SW = Js * CL  # slot logical width
    assert SW % nmm == 0
    nk = SW // nmm  # mm chunks per slot
    assert nk % 4 == 0 or nk == 2
    kgrp = 4 if nk % 4 == 0 else 2  # psum banks per evac group
    assert CL % (2 * stores_per_slot) == 0
    assert Js % cast_sub == 0 and epack in (1, 4)

    nc = bacc.Bacc(trn_type="TRN2", target_bir_lowering=False, debug=False)

    x = nc.declare_dram_parameter("x", [C, F], f32, isOutput=False)
    al = nc.declare_dram_parameter("alpha_rep", [C, 1], f32, isOutput=False)
    sel4 = nc.declare_dram_parameter("sel4", [C, 4 * C], f32, isOutput=False)
    id32 = nc.declare_dram_parameter("ident32", [C, T], f32, isOutput=False)
    # y stored folded: host de-folds (see unfold_y)
    y = nc.declare_dram_parameter("y", [C, F], f32, isOutput=True)

    with ExitStack() as ctx:
        tc = ctx.enter_context(tile.TileContext(nc))
        consts = ctx.enter_context(tc.tile_pool(name="consts", bufs=1))
        smalls = ctx.enter_context(tc.tile_pool(name="smalls", bufs=1))
        xn_pool = ctx.enter_context(tc.tile_pool(name="xn", bufs=1))
        xnbf_pool = ctx.enter_context(tc.tile_pool(name="xnbf", bufs=2))
        qtb_pool = ctx.enter_context(
            tc.tile_pool(name="qtb", bufs=qtb_bufs or (nslot * nk) // kgrp)
        )
        psE_stack = ExitStack()
        psE = psE_stack.enter_context(tc.tile_pool(name="psE", bufs=1, space="PSUM"))

        alpha_sb = consts.tile([C, 1], f32)
        nc.sync.dma_start(alpha_sb[:], al[:])
        sel_sb = consts.tile([C, 4 * C], f32)
        nc.sync.dma_start(sel_sb[:], sel4[:])
        id_sb = consts.tile([C, T], f32)
        nc.sync.dma_start(id_sb[:], id32[:])
        # Warm the Exp activation table early (overlaps with phase-1 DMA).
        warm = consts.tile([C, 1], f32)
        nc.scalar.activation(warm[:], alpha_sb[:], AF.Exp)

        # XNQ = x (native) in cols [0, F) + nspare spare slot regions at [F, ...)
        XNQ = xn_pool.tile([C, F + nspare * SW], f32)
        xn3 = XNQ[:, 0:F].rearrange("p (t h) -> p t h", t=T)
        xn_hwT = XNQ[:, 0:F].rearrange("p (t h) -> p h t", t=T)
        # x arrives slot-major-striped (host: make_in_maps) so every chunk
        # load reads a fully contiguous DRAM range at max HBM efficiency

        def qt_cells(m, cl0, ncl, j0, nj, jmajor=False):
            """AP over qt slot m cells: [p][cl][jj] (or [p][jj][cl])."""
            if m < nspare:
                a0 = F + m * SW
                v = XNQ[:, a0 : a0 + SW].rearrange("p (cl j) -> p cl j", cl=CL)
                v = v[:, cl0 : cl0 + ncl, j0 : j0 + nj]
            else:
                base = (m - nspare) * Js
                v = XNQ[:, 0:F].rearrange("p (cl h) -> p cl h", cl=CL)
                v = v[:, cl0 : cl0 + ncl, base + j0 : base + j0 + nj]
            if jmajor:
                v = v.rearrange("p cl j -> p j cl")
            return v

        cast_eng = {"gpsimd": nc.gpsimd, "scalar": nc.scalar, "vector": nc.vector}[
            cast_engine
        ]

        Bt = smalls.tile([C, T], f32)
        Btb = smalls.tile([C, T], bf16)
        qtbs = {}

        def emit_qtb(m, eng="scalar"):
            for k in range(nk // kgrp):
                qtb = qtb_pool.tile([C, kgrp * nmm], bf16, tag="qtb")
                qtbs[(m, k)] = qtb
                qb = qtb[:].rearrange(
                    "p (b cl2 j) -> p b cl2 j", b=kgrp, cl2=nmm // Js
                )
                src = qt_cells(
                    m, k * kgrp * (nmm // Js), kgrp * (nmm // Js), 0, Js
                ).rearrange("p (b cl2) j -> p b cl2 j", b=kgrp)
                if eng == "gpsimd":
                    nc.gpsimd.tensor_copy(qb, src)
                else:
                    nc.scalar.copy(qb, src)

        # ---- Phase 1: load + cast + energy + transpose-to-folded ----
        EP = T * epack
        P4 = psE.tile([EP, EP], f32)
        nsub = nstripe // nslot
        Jsub = Js // nsub
        for m in range(nslot):
            for hh in range(nsub):
                k = m * nsub + hh
                src = x[:, k * T * Jsub : (k + 1) * T * Jsub].rearrange(
                    "p (t j) -> p t j", t=T
                )
                d0 = m * Js + hh * Jsub
                nc.sync.dma_start(xn3[:, :, d0 : d0 + Jsub], src)
            # slot layout: cell(t, j) = (j//ep)*(T*ep) + t*ep + j%ep, so each
            # energy group (all t, ep consecutive hw) is one contiguous
            # T*ep-column run (single-free-dim matmul weight AP, 256B reads)
            xb = xnbf_pool.tile([C, T * Js], bf16, tag="xnbf")
            ep = epack
            xb4 = xb[:].rearrange("p (jb t j4) -> p t jb j4", t=T, j4=ep)
            sub = Js // cast_sub
            assert sub % ep == 0
            for s in range(cast_sub):
                lo = s * sub
                hi = lo + sub
                o = xb4[:, :, lo // ep : hi // ep, :]
                i = xn3[:, :, m * Js + lo : m * Js + hi].rearrange(
                    "p t (jb j4) -> p t jb j4", j4=ep
                )
                if m == nslot - 1 and cast_engine == "gpsimd" and s >= cast_sub // 2:
                    nc.scalar.copy(o, i)  # split the last chunk's cast tail
                elif cast_engine == "scalar":
                    nc.scalar.copy(o, i)
                else:
                    cast_eng.tensor_copy(o, i)
            for jl in range(0, Js, ep):
                a = xb[:, (jl // ep) * T * ep : (jl // ep + 1) * T * ep]
                gidx = m * (Js // ep) + jl // ep
                nc.tensor.matmul(
                    P4[:],
                    a,
                    a,
                    start=(gidx == 0),
                    stop=(gidx == HB // ep - 1),
                )
            # transpose chunk m into qt slot m (region / spare), per sub-chunk
            if not (defer_last_t and m == nslot - 1):
                for hh in range(nsub):
                    j0 = hh * Jsub
                    nc.vector.transpose(
                        qt_cells(m, 0, CL, j0, Jsub, jmajor=True),
                        xn_hwT[:, m * Js + j0 : m * Js + j0 + Jsub, :],
                    )
            if qtb_ahead and m < nslot - qtb_late:
                emit_qtb(m, "gpsimd" if m in qtb_gp_slots else "scalar")

        # ---- Softmax -> B^T (replicated x4 on partition groups) ----
        P4sb = smalls.tile([EP, EP], f32)
        nc.scalar.copy(P4sb[:], P4[:])
        Erep = psE.tile([C, T], f32)
        if epack == 1:
            nc.tensor.matmul(Erep[:], sel_sb[0:T, 0:C], P4sb[:], start=True, stop=True)
        else:
            p4v = P4sb[:].rearrange("p (s j) -> p s j", j=epack)
            for jj in range(epack):
                nc.tensor.matmul(
                    Erep[:],
                    sel_sb[:, jj * C : (jj + 1) * C],
                    p4v[:, :, jj],
                    start=(jj == 0),
                    stop=(jj == epack - 1),
                )
        negmax = smalls.tile([C, 1], f32)
        nc.vector.tensor_reduce(
            negmax[:], Erep[:], axis=AX.X, op=ALU.max, negate=True
        )
        P = smalls.tile([C, T], f32)
        ssum = smalls.tile([C, 1], f32)
        nc.scalar.activation(
            P[:], Erep[:], AF.Exp, bias=negmax[:], scale=1.0, accum_out=ssum[:]
        )
        rcp = smalls.tile([C, 1], f32)
        nc.vector.reciprocal(rcp[:], ssum[:])
        Bp = smalls.tile([C, T], f32)
        nc.vector.tensor_scalar(
            out=Bp[:],
            in0=P[:],
            scalar1=rcp[:],
            scalar2=alpha_sb[:],
            op0=ALU.mult,
            op1=ALU.mult,
        )
        if mode == "fused":
            nc.vector.tensor_add(Bp[:], Bp[:], id_sb[:])
        nc.vector.transpose(Bt[:], Bp[:])
        nc.vector.tensor_copy(Btb[:], Bt[:])
        psE_stack.close()  # release P4/Erep PSUM banks for phase 2
        if qtb_ahead:
            for m in range(nslot - qtb_late, nslot):
                emit_qtb(m)

        # ---- Phase 2: attention matmul + residual + store ----
        # y is slot-major folded: y[p, m*SW + cl*Js + jj] -> every store
        # writes a contiguous DRAM range (host de-folds, see unfold_y)
        ncl_mm = nmm // Js
        with ExitStack() as p2:
            tmpp = (
                p2.enter_context(tc.tile_pool(name="tmp", bufs=2))
                if gs_num > 0
                else None
            )
            ps2 = p2.enter_context(tc.tile_pool(name="ps2", bufs=2, space="PSUM"))
            evac_idx = 0
            for m in range(nslot):
                if defer_last_t and m == 1:
                    # last slot's transpose runs after slot-0's evacuation,
                    # letting softmax + first stores precede it on DVE
                    mm = nslot - 1
                    nc.vector.transpose(
                        qt_cells(mm, 0, CL, 0, Js, jmajor=True),
                        xn_hwT[:, mm * Js : (mm + 1) * Js, :],
                    )
                if not qtb_ahead:
                    emit_qtb(m)
                for k in range(nk // kgrp):
                    qtb = qtbs[(m, k)]
                    ps = ps2.tile([C, kgrp * 512], f32)
                    for b in range(kgrp):
                        for g in range(G):
                            nc.tensor.matmul(
                                ps[g * 32 : (g + 1) * 32, b * 512 : b * 512 + nmm],
                                Btb[g * 32 : (g + 1) * 32, :],
                                qtb[g * 32 : (g + 1) * 32, b * nmm : (b + 1) * nmm],
                                start=True,
                                stop=True,
                                tile_position=(g * 32, g * 32),
                            )
                    pv = (
                        ps[:]
                        .rearrange("p (b r) -> p b r", b=kgrp)[:, :, 0:nmm]
                        .rearrange("p b (cl2 j) -> p b cl2 j", cl2=ncl_mm)
                    )
                    qv = qt_cells(
                        m, k * kgrp * ncl_mm, kgrp * ncl_mm, 0, Js
                    ).rearrange("p (b cl2) j -> p b cl2 j", b=kgrp)
                    if mode == "fused":
                        nc.scalar.copy(qv, pv)
                    else:
                        use_gp = (evac_idx % gs_den) < gs_num
                        evac_idx += 1
                        if use_gp:
                            tmp = tmpp.tile([C, kgrp * nmm], f32, tag="evac")
                            t3 = tmp[:].rearrange(
                                "p (b cl2 j) -> p b cl2 j", b=kgrp, cl2=ncl_mm
                            )
                            nc.scalar.copy(t3, pv)
                            nc.gpsimd.tensor_add(qv, qv, t3)
                        else:
                            nc.vector.tensor_add(qv, qv, pv)
                # store slot in pieces (cl ranges), contiguous in DRAM
                ncl_st = CL // stores_per_slot
                for s in range(stores_per_slot):
                    sb = qt_cells(m, s * ncl_st, ncl_st, 0, Js)
                    a = m * SW + s * ncl_st * Js
                    dr = y[:, a : a + ncl_st * Js].rearrange(
                        "p (cl j) -> p cl j", cl=ncl_st
                    )
                    nc.sync.dma_start(dr, sb)

    nc.compile()  # bacc passes: reg alloc, wait splitting (1-wait HW limit), ...
    return nc


def _consts():
    # sel4[u*4+jj', 32g+t] for block jj: 1 iff jj'==jj and u==t
    sel = np.zeros((C, 4 * C), np.float32)
    for jj in range(4):
        for t in range(T):
            for g in range(G):
                sel[t * 4 + jj, jj * C + g * 32 + t] = 1.0
    id32 = np.zeros((C, T), np.float32)
    for p in range(C):
        id32[p, p % T] = 1.0
    return sel, id32


_BUILD_KW = dict(mode="exact", nspare=2, qtb_bufs=4)


_NSLOT = 4  # must match build_nc(nslot=...)
_NSTRIPE = 4  # must match build_nc(nstripe=...)


def make_in_maps(x: np.ndarray, alpha: np.ndarray):
    assert x.shape == (N, C, T, H, W) and x.dtype == np.float32
    sel, id32 = _consts()
    alpha_rep = np.full((C, 1), np.float32(alpha.reshape(-1)[0]), np.float32)
    # stripe-major: x_str[p, k*T*Js + t*Js + j] = x[p, t, k*Js + j]
    Js = HB // _NSTRIPE
    xr = np.ascontiguousarray(
        x.reshape(N, C, T, _NSTRIPE, Js).transpose(0, 1, 3, 2, 4).reshape(N, C, F)
    )
    return [
        {"x": xr[n], "alpha_rep": alpha_rep, "sel4": sel, "ident32": id32}
        for n in range(NCORES)
    ]


def kernel(x: np.ndarray, alpha: np.ndarray) -> np.ndarray:
    from concourse.bass_utils import run_bass_kernel_spmd

    nc = build_nc(**_BUILD_KW)
    in_maps = make_in_maps(x, alpha)
    res = run_bass_kernel_spmd(nc, in_maps, list(range(NCORES)))
    out = np.stack([unfold_y(res.results[n]["y"]) for n in range(NCORES)])
    return out.astype(np.float32)


def unfold_y(yf: np.ndarray) -> np.ndarray:
    # yf[32g+t, m*SW + cl*Js + jj] = out[32g+cl, t, m*Js+jj]  ->  (C, T, H, W)
    Js = HB // _NSLOT
    return (
        np.asarray(yf)
        .reshape(G, T, _NSLOT, CL, Js)
        .transpose(0, 3, 1, 2, 4)
        .reshape(C, T, H, W)
    )



# revision 2
# speedup vs baseline: 1.6840x; 1.6840x over previous
"""Trainium2 Bass kernel for nn_AttentionMechanism_21646635172225.

Reference computation (per batch element n):
    q   = transpose(x[n], (T,C,H,W)).reshape(T, C*H*W)      # x[n]: (C,T,H,W)
    E   = q @ q.T                                            # (T, T)
    A   = softmax(E, axis=-1)
    out = alpha * (A @ q) + q          -> reshape/transpose back to (C,T,H,W)

Sharding: data-parallel over batch N=8 across the 8 NeuronCores (one batch
element per core), alpha replicated.

bf16 end-to-end design (rel-err budget 2e-2; bf16 round-trip is ~2e-3):
  - Host pre-casts x to bf16 and pre-packs the energy cell layout
    xcell[c, m, jb, t, j4] = x[c, t, m*Js + jb*4 + j4], so each chunk load
    is one contiguous 12.5KB-per-partition DMA (big packets) and the energy
    matmul weight/moving APs are single contiguous 128-column runs.
  - Energy Gram matrix accumulates via 4-hw-packed bf16 matmuls into PSUM
    P4 (4 diagonal 32x32 blocks summed by bf16 selector matmuls).
  - DVE 32x32 stream-transposes fold each chunk into t-major qt layout.
    bf16 pairs are bitcast to fp32 so the transpose is a plain 4-byte
    32x32 block transpose; the resulting hw-pair interleave is the
    identity (pairs stay adjacent), so qt[32g+t, m*SW + cl*Js + j] =
    x[32g+cl, t, m*Js + j] exactly.
  - Softmax on the replicated [128, 32] energy; residual is FUSED into the
    attention weight: B = alpha*A + I, built as a block-diagonal [128,128]
    bf16 weight W (B^T per 32x32 diagonal block). One weight load serves
    all phase-2 matmuls; out = W^T-contract over all 128 partitions gives
    alpha*(A@q) + q per group. alpha=0 stays bitwise exact (0-weights give
    exact-zero products; 1.0-weights pass qt through).
  - Phase-2 evacuation is a pure scalar copy PSUM fp32 -> SBUF bf16 into a
    contiguous per-chunk store buffer; stores are 6.2KB-per-partition DMAs.
  - y returns folded bf16; host de-folds and upcasts to fp32.

HBM traffic: 6.4MB in + 6.4MB out per core (vs 25.7MB for fp32).
"""

import sys

sys.path.insert(0, "/opt/trn_rl_repo")

from contextlib import ExitStack

import numpy as np

import concourse.bass as bass
import concourse.tile as tile
from concourse import bacc, mybir

# Problem shape (hardcoded per contract)
N, C, T, H, W = 8, 128, 32, 28, 28
HB = H * W  # 784
F = T * HB  # 25088
G = 4  # partition groups (c blocks of 32)
CL = 32  # c-local within group
NCORES = 8

f32 = mybir.dt.float32
bf16 = mybir.dt.bfloat16
AF = mybir.ActivationFunctionType
ALU = mybir.AluOpType
AX = mybir.AxisListType

NSLOT = 4  # chunks
Js = HB // NSLOT  # 196 hw per chunk
SW = T * Js  # 6272 chunk width (bf16 cols)
EP = 4  # hw packed per energy matmul group
JB = Js // EP  # 49 energy groups per chunk
NMM = 392  # phase-2 moving cols per matmul
KGRP = 4  # psum banks per phase-2 tile
NK = SW // NMM  # 16 phase-2 matmuls per chunk


def build_nc(
    nloads: int = 2,  # dma_starts per chunk load
    nstores: int = 2,  # dma_starts per chunk store
    evac_engines: tuple = ("scalar",),  # cycle for PSUM->SBUF evacuation
    defer_last_t: bool = True,  # emit last chunk's transposes after softmax
):
    nc = bacc.Bacc(trn_type="TRN2", target_bir_lowering=False, debug=False)

    x = nc.declare_dram_parameter("x", [C, F], bf16, isOutput=False)
    al = nc.declare_dram_parameter("alpha_rep", [C, 1], f32, isOutput=False)
    sel4 = nc.declare_dram_parameter("sel4", [C, 4 * C], bf16, isOutput=False)
    id32 = nc.declare_dram_parameter("ident32", [C, T], f32, isOutput=False)
    y = nc.declare_dram_parameter("y", [C, F], bf16, isOutput=True)

    with ExitStack() as ctx:
        tc = ctx.enter_context(tile.TileContext(nc))
        consts = ctx.enter_context(tc.tile_pool(name="consts", bufs=1))
        smalls = ctx.enter_context(tc.tile_pool(name="smalls", bufs=1))
        xn_pool = ctx.enter_context(tc.tile_pool(name="xn", bufs=1))
        qt_pool = ctx.enter_context(tc.tile_pool(name="qt", bufs=1))
        psE_stack = ExitStack()
        psE = psE_stack.enter_context(tc.tile_pool(name="psE", bufs=1, space="PSUM"))

        alpha_sb = consts.tile([C, 1], f32)
        nc.sync.dma_start(alpha_sb[:], al[:])
        sel_sb = consts.tile([C, 4 * C], bf16)
        nc.sync.dma_start(sel_sb[:], sel4[:])
        id_sb = consts.tile([C, T], f32)
        nc.sync.dma_start(id_sb[:], id32[:])
        # Warm the Exp activation table early (overlaps with phase-1 DMA).
        warm = consts.tile([C, 1], f32)
        nc.scalar.activation(warm[:], alpha_sb[:], AF.Exp)
        # Block-diagonal phase-2 weight; zero the off-diagonal early (gpsimd).
        W128 = smalls.tile([C, C], bf16)
        nc.gpsimd.memset(W128[:], 0.0)

        XN = xn_pool.tile([C, F], bf16)
        QT = qt_pool.tile([C, F], bf16)

        def emit_transpose(m):
            # fp32-pair 32x32 block transpose: fold chunk m into QT
            inf = (
                XN[:, m * SW : (m + 1) * SW]
                .bitcast(f32)
                .rearrange("p (jb t two) -> p jb two t", t=T, two=2)
            )
            outf = (
                QT[:, m * SW : (m + 1) * SW]
                .bitcast(f32)
                .rearrange("p (cl jb two) -> p jb two cl", cl=CL, two=2)
            )
            for j4p in range(2):
                nc.vector.transpose(
                    outf[:, :, j4p, :], inf[:, :, j4p, :]
                )

        # ---- Phase 1: load + energy + transpose-to-folded ----
        P4 = psE.tile([C, C], f32)
        LD = SW // nloads
        for m in range(NSLOT):
            for h in range(nloads):
                a0 = m * SW + h * LD
                nc.sync.dma_start(XN[:, a0 : a0 + LD], x[:, a0 : a0 + LD])
            for jb in range(JB):
                a = XN[:, m * SW + jb * (T * EP) : m * SW + (jb + 1) * (T * EP)]
                gidx = m * JB + jb
                nc.tensor.matmul(
                    P4[:],
                    a,
                    a,
                    start=(gidx == 0),
                    stop=(gidx == NSLOT * JB - 1),
                )
            if not (defer_last_t and m == NSLOT - 1):
                emit_transpose(m)

        # ---- Softmax -> W128 (block-diag B^T, B = alpha*A + I) ----
        P4sb = smalls.tile([C, C], bf16)
        nc.scalar.copy(P4sb[:], P4[:])
        Erep = psE.tile([C, T], f32)
        p4v = P4sb[:].rearrange("p (s j) -> p s j", j=EP)
        for jj in range(EP):
            nc.tensor.matmul(
                Erep[:],
                sel_sb[:, jj * C : (jj + 1) * C],
                p4v[:, :, jj],
                start=(jj == 0),
                stop=(jj == EP - 1),
            )
        negmax = smalls.tile([C, 1], f32)
        nc.vector.tensor_reduce(
            negmax[:], Erep[:], axis=AX.X, op=ALU.max, negate=True
        )
        P = smalls.tile([C, T], f32)
        ssum = smalls.tile([C, 1], f32)
        nc.scalar.activation(
            P[:], Erep[:], AF.Exp, bias=negmax[:], scale=1.0, accum_out=ssum[:]
        )
        rcp = smalls.tile([C, 1], f32)
        nc.vector.reciprocal(rcp[:], ssum[:])
        Bp = smalls.tile([C, T], f32)
        nc.vector.tensor_scalar(
            out=Bp[:],
            in0=P[:],
            scalar1=rcp[:],
            scalar2=alpha_sb[:],
            op0=ALU.mult,
            op1=ALU.mult,
        )
        nc.vector.tensor_add(Bp[:], Bp[:], id_sb[:])
        Bt = smalls.tile([C, T], f32)
        nc.vector.transpose(Bt[:], Bp[:])
        for g in range(G):
            nc.scalar.copy(
                W128[g * CL : (g + 1) * CL, g * CL : (g + 1) * CL],
                Bt[g * CL : (g + 1) * CL, :],
            )
        if defer_last_t:
            emit_transpose(NSLOT - 1)
        psE_stack.close()  # release P4/Erep PSUM banks for phase 2

        # ---- Phase 2: fused attention+residual matmul + store ----
        n_evac = 0
        ST = SW // nstores
        with ExitStack() as p2:
            ps2 = p2.enter_context(tc.tile_pool(name="ps2", bufs=2, space="PSUM"))
            ysb_pool = p2.enter_context(tc.tile_pool(name="ysb", bufs=2))
            for m in range(NSLOT):
                ysb = ysb_pool.tile([C, SW], bf16, tag="ysb")
                stores_done = 0
                for kb in range(NK // KGRP):
                    ps = ps2.tile([C, KGRP * 512], f32)
                    for b in range(KGRP):
                        col0 = m * SW + (kb * KGRP + b) * NMM
                        nc.tensor.matmul(
                            ps[:, b * 512 : b * 512 + NMM],
                            W128[:],
                            QT[:, col0 : col0 + NMM],
                            start=True,
                            stop=True,
                        )
                    eng = {
                        "scalar": nc.scalar,
                        "vector": nc.vector,
                        "gpsimd": nc.gpsimd,
                    }[evac_engines[n_evac % len(evac_engines)]]
                    n_evac += 1
                    dst = ysb[:, kb * KGRP * NMM : (kb + 1) * KGRP * NMM].rearrange(
                        "p (b j) -> p b j", b=KGRP
                    )
                    src = ps[:].rearrange("p (b r) -> p b r", b=KGRP)[:, :, 0:NMM]
                    if eng is nc.scalar:
                        nc.scalar.copy(dst, src)
                    else:
                        eng.tensor_copy(dst, src)
                    # store as soon as a full 1/nstores of the chunk is ready
                    done_cols = (kb + 1) * KGRP * NMM
                    while (
                        stores_done < nstores
                        and done_cols >= (stores_done + 1) * ST
                    ):
                        a0 = stores_done * ST
                        nc.sync.dma_start(
                            y[:, m * SW + a0 : m * SW + a0 + ST],
                            ysb[:, a0 : a0 + ST],
                        )
                        stores_done += 1

    nc.compile()
    return nc


def _consts():
    # sel4[u*4+jj', 32g+t] for block jj: 1 iff jj'==jj and u==t
    sel = np.zeros((C, 4 * C), np.float32)
    for jj in range(4):
        for t in range(T):
            for g in range(G):
                sel[t * 4 + jj, jj * C + g * 32 + t] = 1.0
    id32 = np.zeros((C, T), np.float32)
    for p in range(C):
        id32[p, p % T] = 1.0
    return sel, id32


_BUILD_KW = dict()


def make_in_maps(x: np.ndarray, alpha: np.ndarray):
    import ml_dtypes

    assert x.shape == (N, C, T, H, W) and x.dtype == np.float32
    sel, id32 = _consts()
    sel_bf = sel.astype(ml_dtypes.bfloat16)
    alpha_rep = np.full((C, 1), np.float32(alpha.reshape(-1)[0]), np.float32)
    # energy cell layout: xc[c, m, jb, t, j4] = x[c, t, m*Js + jb*4 + j4]
    xr = (
        x.reshape(N, C, T, NSLOT, JB, EP)
        .transpose(0, 1, 3, 4, 2, 5)
        .reshape(N, C, F)
        .astype(ml_dtypes.bfloat16)
    )
    xr = np.ascontiguousarray(xr)
    return [
        {"x": xr[n], "alpha_rep": alpha_rep, "sel4": sel_bf, "ident32": id32}
        for n in range(NCORES)
    ]


def unfold_y(yf: np.ndarray) -> np.ndarray:
    # yf[32g+t, m*SW + cl*Js + j] = out[32g+cl, t, m*Js+j]  ->  (C, T, H, W)
    return (
        np.asarray(yf)
        .astype(np.float32)
        .reshape(G, T, NSLOT, CL, Js)
        .transpose(0, 3, 1, 2, 4)
        .reshape(C, T, H, W)
    )


def kernel(x: np.ndarray, alpha: np.ndarray) -> np.ndarray:
    from concourse.bass_utils import run_bass_kernel_spmd

    nc = build_nc(**_BUILD_KW)
    in_maps = make_in_maps(x, alpha)
    res = run_bass_kernel_spmd(nc, in_maps, list(range(NCORES)))
    out = np.stack([unfold_y(res.results[n]["y"]) for n in range(NCORES)])
    return out.astype(np.float32)


# revision 4
# speedup vs baseline: 1.7478x; 1.0379x over previous
"""Trainium2 Bass kernel for nn_AttentionMechanism_21646635172225.

Reference computation (per batch element n):
    q   = transpose(x[n], (T,C,H,W)).reshape(T, C*H*W)      # x[n]: (C,T,H,W)
    E   = q @ q.T                                            # (T, T)
    A   = softmax(E, axis=-1)
    out = alpha * (A @ q) + q          -> reshape/transpose back to (C,T,H,W)

Sharding: data-parallel over batch N=8 across the 8 NeuronCores (one batch
element per core), alpha replicated.

bf16 end-to-end design (rel-err budget 2e-2; bf16 round-trip is ~2e-3):
  - Host pre-casts x to bf16 and pre-packs the energy cell layout
    xcell[c, m, jb, j4p, t, e] = x[c, t, m*Js + jb*4 + j4p*2 + e]: each
    chunk load is one contiguous 7KB-per-partition DMA and each energy
    matmul group (fixed jb) is a single contiguous 128-column run.
  - Energy Gram matrix accumulates via 4-hw-packed bf16 matmuls into PSUM
    P4; the 4 stride-2 diagonal 32x32 sub-blocks (one per (j4p, e)) are
    summed and replicated to the 4 partition groups by bf16 selector
    matmuls.
  - DVE 32x32 stream-transposes fold each chunk into the t-major qt layout
    qt[32g+t, m*SW + jh*64 + cl*2 + e] = x[32g+cl, t, m*Js + jh*2 + e].
    bf16 hw-pairs are bitcast to fp32 so this is a plain 4-byte 32x32
    block transpose, and both the input AP (jb, t) and output AP (jh, cl)
    are stride-1 in their innermost dim (full 1 elem/cycle/lane rate).
  - Softmax on the replicated [128, 32] energy; residual is FUSED into the
    attention weight: B = alpha*A + I, built as a block-diagonal [128,128]
    bf16 weight W (B^T per 32x32 diagonal block). One weight load serves
    all phase-2 matmuls; the full-partition contraction with block-diag W
    gives alpha*(A@q) + q per group. alpha=0 stays bitwise exact.
  - Phase-2 evacuation is a pure copy PSUM fp32 -> SBUF bf16, round-robin
    across scalar/vector/gpsimd; stores are 3.5KB-per-partition DMAs from
    a contiguous store buffer. y returns folded bf16; host de-folds and
    upcasts.

HBM traffic: 6.4MB in + 6.4MB out per core (vs 25.7MB for fp32).
"""

import sys

sys.path.insert(0, "/opt/trn_rl_repo")

from contextlib import ExitStack

import numpy as np

import concourse.bass as bass
import concourse.tile as tile
from concourse import bacc, mybir

# Problem shape (hardcoded per contract)
N, C, T, H, W = 8, 128, 32, 28, 28
HB = H * W  # 784
F = T * HB  # 25088
G = 4  # partition groups (c blocks of 32)
CL = 32  # c-local within group
NCORES = 8

f32 = mybir.dt.float32
bf16 = mybir.dt.bfloat16
AF = mybir.ActivationFunctionType
ALU = mybir.AluOpType
AX = mybir.AxisListType

NSLOT = 7  # chunks
Js = HB // NSLOT  # 112 hw per chunk
SW = T * Js  # 3584 chunk width (bf16 cols)
EP = 4  # hw packed per energy matmul group
JB = Js // EP  # 28 energy groups per chunk
NMM = 448  # phase-2 moving cols per matmul
KGRP = 4  # psum banks per phase-2 tile
NK = SW // NMM  # 8 phase-2 matmuls per chunk


def build_nc(
    nloads: int = 1,  # dma_starts per chunk load
    evac_engines: tuple = ("scalar", "vector"),  # gpsimd cannot read PSUM
    defer_last_t: bool = False,
):
    nc = bacc.Bacc(trn_type="TRN2", target_bir_lowering=False, debug=False)

    x = nc.declare_dram_parameter("x", [C, F], bf16, isOutput=False)
    al = nc.declare_dram_parameter("alpha_rep", [C, 1], f32, isOutput=False)
    sel4 = nc.declare_dram_parameter("sel4", [C, 4 * C], bf16, isOutput=False)
    id32 = nc.declare_dram_parameter("ident32", [C, T], f32, isOutput=False)
    y = nc.declare_dram_parameter("y", [C, F], bf16, isOutput=True)

    with ExitStack() as ctx:
        tc = ctx.enter_context(tile.TileContext(nc))
        consts = ctx.enter_context(tc.tile_pool(name="consts", bufs=1))
        smalls = ctx.enter_context(tc.tile_pool(name="smalls", bufs=1))
        xn_pool = ctx.enter_context(tc.tile_pool(name="xn", bufs=1))
        qt_pool = ctx.enter_context(tc.tile_pool(name="qt", bufs=1))
        psE_stack = ExitStack()
        psE = psE_stack.enter_context(tc.tile_pool(name="psE", bufs=1, space="PSUM"))

        # consts go through the gpsimd DGE queue so the x stream starts
        # immediately on the sync queue
        W128 = smalls.tile([C, C], bf16)
        nc.gpsimd.memset(W128[:], 0.0)
        alpha_sb = consts.tile([C, 1], f32)
        nc.gpsimd.dma_start(alpha_sb[:], al[:])
        sel_sb = consts.tile([C, 4 * C], bf16)
        nc.gpsimd.dma_start(sel_sb[:], sel4[:])
        id_sb = consts.tile([C, T], f32)
        nc.gpsimd.dma_start(id_sb[:], id32[:])
        # Warm the Exp activation table early (overlaps with phase-1 DMA).
        warm = consts.tile([C, 1], f32)
        nc.scalar.activation(warm[:], alpha_sb[:], AF.Exp)

        XN = xn_pool.tile([C, F], bf16)
        QT = qt_pool.tile([C, F], bf16)

        def emit_transpose(m):
            # fp32-pair 32x32 block transpose: fold chunk m into QT.
            # in cells (jb, j4p, t) fp32; out cells (jh=2jb+j4p, cl) fp32.
            inf = (
                XN[:, m * SW : (m + 1) * SW]
                .bitcast(f32)
                .rearrange("p (jb j4p t) -> p jb j4p t", t=T, j4p=2)
            )
            outf = (
                QT[:, m * SW : (m + 1) * SW]
                .bitcast(f32)
                .rearrange("p (jb j4p cl) -> p jb j4p cl", cl=CL, j4p=2)
            )
            for j4p in range(2):
                nc.vector.transpose(outf[:, :, j4p, :], inf[:, :, j4p, :])

        # ---- Phase 1: load + energy + transpose-to-folded ----
        P4 = psE.tile([C, C], f32)
        LD = SW // nloads
        for m in range(NSLOT):
            for h in range(nloads):
                a0 = m * SW + h * LD
                nc.sync.dma_start(XN[:, a0 : a0 + LD], x[:, a0 : a0 + LD])
            for jb in range(JB):
                a = XN[:, m * SW + jb * (T * EP) : m * SW + (jb + 1) * (T * EP)]
                gidx = m * JB + jb
                nc.tensor.matmul(
                    P4[:],
                    a,
                    a,
                    start=(gidx == 0),
                    stop=(gidx == NSLOT * JB - 1),
                )
            if not (defer_last_t and m == NSLOT - 1):
                emit_transpose(m)

        # ---- Softmax -> W128 (block-diag B^T, B = alpha*A + I) ----
        P4sb = smalls.tile([C, C], bf16)
        nc.scalar.copy(P4sb[:], P4[:])
        Erep = psE.tile([C, T], f32)
        # P4 cols are (j4p, t, e); block (j4p, e) holds a stride-2 diagonal
        p4v = P4sb[:].rearrange("p (a t b) -> p a b t", a=2, b=2)
        for jj in range(EP):
            nc.tensor.matmul(
                Erep[:],
                sel_sb[:, jj * C : (jj + 1) * C],
                p4v[:, jj >> 1, jj & 1, :],
                start=(jj == 0),
                stop=(jj == EP - 1),
            )
        negmax = smalls.tile([C, 1], f32)
        nc.vector.tensor_reduce(
            negmax[:], Erep[:], axis=AX.X, op=ALU.max, negate=True
        )
        P = smalls.tile([C, T], f32)
        ssum = smalls.tile([C, 1], f32)
        nc.scalar.activation(
            P[:], Erep[:], AF.Exp, bias=negmax[:], scale=1.0, accum_out=ssum[:]
        )
        rcp = smalls.tile([C, 1], f32)
        nc.vector.reciprocal(rcp[:], ssum[:])
        Bp = smalls.tile([C, T], f32)
        nc.vector.tensor_scalar(
            out=Bp[:],
            in0=P[:],
            scalar1=rcp[:],
            scalar2=alpha_sb[:],
            op0=ALU.mult,
            op1=ALU.mult,
        )
        nc.vector.tensor_add(Bp[:], Bp[:], id_sb[:])
        Bt = smalls.tile([C, T], f32)
        nc.vector.transpose(Bt[:], Bp[:])
        for g in range(G):
            nc.scalar.copy(
                W128[g * CL : (g + 1) * CL, g * CL : (g + 1) * CL],
                Bt[g * CL : (g + 1) * CL, :],
            )
        if defer_last_t:
            emit_transpose(NSLOT - 1)
        psE_stack.close()  # release P4/Erep PSUM banks for phase 2

        # ---- Phase 2: fused attention+residual matmul + store ----
        n_evac = 0
        with ExitStack() as p2:
            ps2 = p2.enter_context(tc.tile_pool(name="ps2", bufs=2, space="PSUM"))
            ysb_pool = p2.enter_context(tc.tile_pool(name="ysb", bufs=2))
            for m in range(NSLOT):
                ysb = ysb_pool.tile([C, SW], bf16, tag="ysb")
                for kb in range(NK // KGRP):
                    ps = ps2.tile([C, KGRP * 512], f32)
                    for b in range(KGRP):
                        col0 = m * SW + (kb * KGRP + b) * NMM
                        nc.tensor.matmul(
                            ps[:, b * 512 : b * 512 + NMM],
                            W128[:],
                            QT[:, col0 : col0 + NMM],
                            start=True,
                            stop=True,
                        )
                    eng = {
                        "scalar": nc.scalar,
                        "vector": nc.vector,
                        "gpsimd": nc.gpsimd,
                    }[evac_engines[n_evac % len(evac_engines)]]
                    n_evac += 1
                    a0 = kb * KGRP * NMM
                    dst = ysb[:, a0 : a0 + KGRP * NMM].rearrange(
                        "p (b j) -> p b j", b=KGRP
                    )
                    src = ps[:].rearrange("p (b r) -> p b r", b=KGRP)[:, :, 0:NMM]
                    if eng is nc.scalar:
                        nc.scalar.copy(dst, src)
                    else:
                        eng.tensor_copy(dst, src)
                    # store each evacuated half-chunk immediately
                    nc.sync.dma_start(
                        y[:, m * SW + a0 : m * SW + a0 + KGRP * NMM],
                        ysb[:, a0 : a0 + KGRP * NMM],
                    )

    nc.compile()
    return nc


def _consts():
    # P4 rows are (j4p, t, e); selector block jj=(j4p, e) extracts that
    # stride-2 diagonal and replicates it to all 4 partition groups:
    # sel[64*j4p + 2*t + e, (2*j4p+e)*C + 32*g + t] = 1
    sel = np.zeros((C, 4 * C), np.float32)
    for j4p in range(2):
        for e in range(2):
            jj = 2 * j4p + e
            for t in range(T):
                for g in range(G):
                    sel[64 * j4p + 2 * t + e, jj * C + g * 32 + t] = 1.0
    id32 = np.zeros((C, T), np.float32)
    for p in range(C):
        id32[p, p % T] = 1.0
    return sel, id32


_BUILD_KW = dict()


def make_in_maps(x: np.ndarray, alpha: np.ndarray):
    import ml_dtypes

    assert x.shape == (N, C, T, H, W) and x.dtype == np.float32
    sel, id32 = _consts()
    sel_bf = sel.astype(ml_dtypes.bfloat16)
    alpha_rep = np.full((C, 1), np.float32(alpha.reshape(-1)[0]), np.float32)
    # energy cell layout: xc[c, m, jb, j4p, t, e] = x[c, t, m*Js+jb*4+j4p*2+e]
    xr = (
        x.reshape(N, C, T, NSLOT, JB, 2, 2)
        .transpose(0, 1, 3, 4, 5, 2, 6)
        .reshape(N, C, F)
        .astype(ml_dtypes.bfloat16)
    )
    xr = np.ascontiguousarray(xr)
    return [
        {"x": xr[n], "alpha_rep": alpha_rep, "sel4": sel_bf, "ident32": id32}
        for n in range(NCORES)
    ]


def unfold_y(yf: np.ndarray) -> np.ndarray:
    # yf[32g+t, m*SW + jh*64 + cl*2 + e] = out[32g+cl, t, m*Js + jh*2 + e]
    return (
        np.asarray(yf)
        .astype(np.float32)
        .reshape(G, T, NSLOT, Js // 2, CL, 2)
        .transpose(0, 4, 1, 2, 3, 5)
        .reshape(C, T, H, W)
    )


def kernel(x: np.ndarray, alpha: np.ndarray) -> np.ndarray:
    from concourse.bass_utils import run_bass_kernel_spmd

    nc = build_nc(**_BUILD_KW)
    in_maps = make_in_maps(x, alpha)
    res = run_bass_kernel_spmd(nc, in_maps, list(range(NCORES)))
    out = np.stack([unfold_y(res.results[n]["y"]) for n in range(NCORES)])
    return out.astype(np.float32)


# revision 9
# speedup vs baseline: 1.9831x; 1.1346x over previous
"""Trainium2 Bass kernel for nn_AttentionMechanism_21646635172225.

Reference computation (per batch element n):
    q   = transpose(x[n], (T,C,H,W)).reshape(T, C*H*W)      # x[n]: (C,T,H,W)
    E   = q @ q.T                                            # (T, T)
    A   = softmax(E, axis=-1)
    out = alpha * (A @ q) + q          -> reshape/transpose back to (C,T,H,W)

Sharding: data-parallel over batch N=8 across the 8 NeuronCores (one batch
element per core), alpha replicated.

bf16 end-to-end design (rel-err budget 2e-2; bf16 round-trip is ~2e-3):
  - Host pre-casts x to bf16 and pre-packs the energy cell layout
    xcell[c, m, jb, j4p, t, e] = x[c, t, m*Js + jb*4 + j4p*2 + e]: each
    chunk load is one contiguous 7KB-per-partition DMA and each energy
    matmul group (fixed jb) is a single contiguous 128-column run.
  - Energy Gram matrix accumulates via 4-hw-packed bf16 matmuls into PSUM
    P4; the 4 stride-2 diagonal 32x32 sub-blocks (one per (j4p, e)) are
    summed and replicated to the 4 partition groups by bf16 selector
    matmuls.
  - DVE 32x32 stream-transposes fold each chunk into the t-major qt layout
    qt[32g+t, m*SW + jh*64 + cl*2 + e] = x[32g+cl, t, m*Js + jh*2 + e].
    bf16 hw-pairs are bitcast to fp32 so this is a plain 4-byte 32x32
    block transpose, and both the input AP (jb, t) and output AP (jh, cl)
    are stride-1 in their innermost dim (full 1 elem/cycle/lane rate).
  - Softmax on the replicated [128, 32] energy; residual is FUSED into the
    attention weight: B = alpha*A + I, built as a block-diagonal [128,128]
    bf16 weight W (B^T per 32x32 diagonal block). One weight load serves
    all phase-2 matmuls; the full-partition contraction with block-diag W
    gives alpha*(A@q) + q per group. alpha=0 stays bitwise exact.
  - Phase-2 evacuation is a pure copy PSUM fp32 -> SBUF bf16, round-robin
    across scalar/vector/gpsimd; stores are 3.5KB-per-partition DMAs from
    a contiguous store buffer. y returns folded bf16; host de-folds and
    upcasts.

HBM traffic: 6.4MB in + 6.4MB out per core (vs 25.7MB for fp32).
"""

import sys

sys.path.insert(0, "/opt/trn_rl_repo")

from contextlib import ExitStack

import numpy as np

import concourse.bass as bass
import concourse.tile as tile
from concourse import bacc, mybir

# Problem shape (hardcoded per contract)
N, C, T, H, W = 8, 128, 32, 28, 28
HB = H * W  # 784
F = T * HB  # 25088
G = 4  # partition groups (c blocks of 32)
CL = 32  # c-local within group
NCORES = 8

f32 = mybir.dt.float32
bf16 = mybir.dt.bfloat16
AF = mybir.ActivationFunctionType
ALU = mybir.AluOpType
AX = mybir.AxisListType

NSLOT = 7  # chunks
Js = HB // NSLOT  # 112 hw per chunk
SW = T * Js  # 3584 chunk width (bf16 cols)
EP = 4  # hw packed per energy matmul group
JB = Js // EP  # 28 energy groups per chunk
NMM = 448  # phase-2 moving cols per matmul
KGRP = 2  # psum banks per phase-2 tile
NK = SW // NMM  # 8 phase-2 matmuls per chunk


def build_nc(
    nloads: int = 1,  # dma_starts per chunk load
    evac_engines: tuple = ("scalar", "vector"),  # gpsimd cannot read PSUM
    defer_last_t: bool = False,
):
    nc = bacc.Bacc(trn_type="TRN2", target_bir_lowering=False, debug=False)

    x = nc.declare_dram_parameter("x", [C, F], bf16, isOutput=False)
    al = nc.declare_dram_parameter("alpha_rep", [C, 1], f32, isOutput=False)
    sel4 = nc.declare_dram_parameter("sel4", [C, 4 * C], bf16, isOutput=False)
    id32 = nc.declare_dram_parameter("ident32", [C, T], f32, isOutput=False)
    y = nc.declare_dram_parameter("y", [C, F], bf16, isOutput=True)

    with ExitStack() as ctx:
        tc = ctx.enter_context(tile.TileContext(nc))
        consts = ctx.enter_context(tc.tile_pool(name="consts", bufs=1))
        smalls = ctx.enter_context(tc.tile_pool(name="smalls", bufs=1))
        xn_pool = ctx.enter_context(tc.tile_pool(name="xn", bufs=1))
        qt_pool = ctx.enter_context(tc.tile_pool(name="qt", bufs=1))
        psE_stack = ExitStack()
        psE = psE_stack.enter_context(tc.tile_pool(name="psE", bufs=1, space="PSUM"))

        W128 = smalls.tile([C, C], bf16)
        nc.gpsimd.memset(W128[:], 0.0)
        alpha_sb = consts.tile([C, 1], f32)
        sel_sb = consts.tile([C, 4 * C], bf16)
        id_sb = consts.tile([C, T], f32)
        warm = consts.tile([C, 1], f32)

        XN = xn_pool.tile([C, F], bf16)
        QT = qt_pool.tile([C, F], bf16)

        def emit_const_loads():
            # issued on the sync queue AFTER the x chunk issues (DMA issues
            # are async; consts are only needed from the softmax onwards)
            nc.sync.dma_start(alpha_sb[:], al[:])
            nc.sync.dma_start(sel_sb[:], sel4[:])
            nc.sync.dma_start(id_sb[:], id32[:])
            # Warm the Exp activation table (overlaps with phase-1 DMA).
            nc.scalar.activation(warm[:], alpha_sb[:], AF.Exp)

        def emit_transpose(m):
            # fp32-pair 32x32 block transpose: fold chunk m into QT.
            # in cells (jb, j4p, t) fp32; out cells (jh=2jb+j4p, cl) fp32.
            inf = (
                XN[:, m * SW : (m + 1) * SW]
                .bitcast(f32)
                .rearrange("p (jb j4p t) -> p jb j4p t", t=T, j4p=2)
            )
            outf = (
                QT[:, m * SW : (m + 1) * SW]
                .bitcast(f32)
                .rearrange("p (jb j4p cl) -> p jb j4p cl", cl=CL, j4p=2)
            )
            for j4p in range(2):
                nc.vector.transpose(outf[:, :, j4p, :], inf[:, :, j4p, :])

        # ---- Phase 1: load + energy + transpose-to-folded ----
        P4 = psE.tile([C, C], f32)
        LD = SW // nloads
        for m in range(NSLOT):
            for h in range(nloads):
                a0 = m * SW + h * LD
                nc.sync.dma_start(XN[:, a0 : a0 + LD], x[:, a0 : a0 + LD])
            if m == NSLOT - 1:
                emit_const_loads()
            for jb in range(JB):
                a = XN[:, m * SW + jb * (T * EP) : m * SW + (jb + 1) * (T * EP)]
                gidx = m * JB + jb
                nc.tensor.matmul(
                    P4[:],
                    a,
                    a,
                    start=(gidx == 0),
                    stop=(gidx == NSLOT * JB - 1),
                )
            if not (defer_last_t and m == NSLOT - 1):
                emit_transpose(m)

        # ---- Softmax -> W128 (block-diag B^T, B = alpha*A + I) ----
        P4sb = smalls.tile([C, C], bf16)
        nc.scalar.copy(P4sb[:], P4[:])
        Erep = psE.tile([C, T], f32)
        # P4 cols are (j4p, t, e); block (j4p, e) holds a stride-2 diagonal
        p4v = P4sb[:].rearrange("p (a t b) -> p a b t", a=2, b=2)
        for jj in range(EP):
            nc.tensor.matmul(
                Erep[:],
                sel_sb[:, jj * C : (jj + 1) * C],
                p4v[:, jj >> 1, jj & 1, :],
                start=(jj == 0),
                stop=(jj == EP - 1),
            )
        negmax = smalls.tile([C, 1], f32)
        nc.vector.tensor_reduce(
            negmax[:], Erep[:], axis=AX.X, op=ALU.max, negate=True
        )
        P = smalls.tile([C, T], f32)
        ssum = smalls.tile([C, 1], f32)
        nc.scalar.activation(
            P[:], Erep[:], AF.Exp, bias=negmax[:], scale=1.0, accum_out=ssum[:]
        )
        rcp = smalls.tile([C, 1], f32)
        nc.vector.reciprocal(rcp[:], ssum[:])
        Bp = smalls.tile([C, T], f32)
        nc.vector.tensor_scalar(
            out=Bp[:],
            in0=P[:],
            scalar1=rcp[:],
            scalar2=alpha_sb[:],
            op0=ALU.mult,
            op1=ALU.mult,
        )
        nc.vector.tensor_add(Bp[:], Bp[:], id_sb[:])
        Bt = smalls.tile([C, T], f32)
        nc.vector.transpose(Bt[:], Bp[:])
        for g in range(G):
            eng = nc.scalar if g % 2 == 0 else nc.gpsimd
            blk = (
                W128[g * CL : (g + 1) * CL, g * CL : (g + 1) * CL],
                Bt[g * CL : (g + 1) * CL, :],
            )
            if eng is nc.scalar:
                nc.scalar.copy(*blk)
            else:
                nc.gpsimd.tensor_copy(*blk)
        if defer_last_t:
            emit_transpose(NSLOT - 1)
        psE_stack.close()  # release P4/Erep PSUM banks for phase 2

        # ---- Phase 2: fused attention+residual matmul + store ----
        n_evac = 0
        with ExitStack() as p2:
            ps2 = p2.enter_context(tc.tile_pool(name="ps2", bufs=2, space="PSUM"))
            ysb_pool = p2.enter_context(tc.tile_pool(name="ysb", bufs=2))
            for m in range(NSLOT):
                ysb = ysb_pool.tile([C, SW], bf16, tag="ysb")
                for kb in range(NK // KGRP):
                    ps = ps2.tile([C, KGRP * 512], f32)
                    for b in range(KGRP):
                        col0 = m * SW + (kb * KGRP + b) * NMM
                        nc.tensor.matmul(
                            ps[:, b * 512 : b * 512 + NMM],
                            W128[:],
                            QT[:, col0 : col0 + NMM],
                            start=True,
                            stop=True,
                        )
                    eng = {
                        "scalar": nc.scalar,
                        "vector": nc.vector,
                        "gpsimd": nc.gpsimd,
                    }[evac_engines[n_evac % len(evac_engines)]]
                    n_evac += 1
                    a0 = kb * KGRP * NMM
                    dst = ysb[:, a0 : a0 + KGRP * NMM].rearrange(
                        "p (b j) -> p b j", b=KGRP
                    )
                    src = ps[:].rearrange("p (b r) -> p b r", b=KGRP)[:, :, 0:NMM]
                    if eng is nc.scalar:
                        nc.scalar.copy(dst, src)
                    else:
                        eng.tensor_copy(dst, src)
                    # store per 2 evac tiles (1792 cols -> 3.5KB packets)
                    if kb % 2 == 1:
                        s0 = (kb - 1) * KGRP * NMM
                        nc.sync.dma_start(
                            y[:, m * SW + s0 : m * SW + s0 + 2 * KGRP * NMM],
                            ysb[:, s0 : s0 + 2 * KGRP * NMM],
                        )

    nc.compile()
    return nc


def _consts():
    # P4 rows are (j4p, t, e); selector block jj=(j4p, e) extracts that
    # stride-2 diagonal and replicates it to all 4 partition groups:
    # sel[64*j4p + 2*t + e, (2*j4p+e)*C + 32*g + t] = 1
    sel = np.zeros((C, 4 * C), np.float32)
    for j4p in range(2):
        for e in range(2):
            jj = 2 * j4p + e
            for t in range(T):
                for g in range(G):
                    sel[64 * j4p + 2 * t + e, jj * C + g * 32 + t] = 1.0
    id32 = np.zeros((C, T), np.float32)
    for p in range(C):
        id32[p, p % T] = 1.0
    return sel, id32


_BUILD_KW = dict()


def make_in_maps(x: np.ndarray, alpha: np.ndarray):
    import ml_dtypes

    assert x.shape == (N, C, T, H, W) and x.dtype == np.float32
    sel, id32 = _consts()
    sel_bf = sel.astype(ml_dtypes.bfloat16)
    alpha_rep = np.full((C, 1), np.float32(alpha.reshape(-1)[0]), np.float32)
    # energy cell layout: xc[c, m, jb, j4p, t, e] = x[c, t, m*Js+jb*4+j4p*2+e]
    xr = (
        x.reshape(N, C, T, NSLOT, JB, 2, 2)
        .transpose(0, 1, 3, 4, 5, 2, 6)
        .reshape(N, C, F)
        .astype(ml_dtypes.bfloat16)
    )
    xr = np.ascontiguousarray(xr)
    return [
        {"x": xr[n], "alpha_rep": alpha_rep, "sel4": sel_bf, "ident32": id32}
        for n in range(NCORES)
    ]


def unfold_y(yf: np.ndarray) -> np.ndarray:
    # yf[32g+t, m*SW + jh*64 + cl*2 + e] = out[32g+cl, t, m*Js + jh*2 + e]
    return (
        np.asarray(yf)
        .astype(np.float32)
        .reshape(G, T, NSLOT, Js // 2, CL, 2)
        .transpose(0, 4, 1, 2, 3, 5)
        .reshape(C, T, H, W)
    )


def kernel(x: np.ndarray, alpha: np.ndarray) -> np.ndarray:
    from concourse.bass_utils import run_bass_kernel_spmd

    nc = build_nc(**_BUILD_KW)
    in_maps = make_in_maps(x, alpha)
    res = run_bass_kernel_spmd(nc, in_maps, list(range(NCORES)))
    out = np.stack([unfold_y(res.results[n]["y"]) for n in range(NCORES)])
    return out.astype(np.float32)


# revision 10
# speedup vs baseline: 2.2065x; 1.1126x over previous
"""Trainium2 Bass kernel for nn_AttentionMechanism_21646635172225.

Reference computation (per batch element n):
    q   = transpose(x[n], (T,C,H,W)).reshape(T, C*H*W)      # x[n]: (C,T,H,W)
    E   = q @ q.T                                            # (T, T)
    A   = softmax(E, axis=-1)
    out = alpha * (A @ q) + q          -> reshape/transpose back to (C,T,H,W)

Sharding: data-parallel over batch N=8 across the 8 NeuronCores (one batch
element per core), alpha replicated.

bf16 end-to-end design (rel-err budget 2e-2; bf16 round-trip is ~2e-3):
  - Host pre-casts x to bf16 and pre-packs the energy cell layout
    xcell[c, m, jb, j4p, t, e] = x[c, t, m*Js + jb*4 + j4p*2 + e]: each
    chunk load is one contiguous 7KB-per-partition DMA and each energy
    matmul group (fixed jb) is a single contiguous 128-column run.
  - Energy Gram matrix accumulates via 4-hw-packed bf16 matmuls into PSUM
    P4; the 4 stride-2 diagonal 32x32 sub-blocks (one per (j4p, e)) are
    summed and replicated to the 4 partition groups by bf16 selector
    matmuls.
  - DVE 32x32 stream-transposes fold each chunk into the t-major qt layout
    qt[32g+t, m*SW + jh*64 + cl*2 + e] = x[32g+cl, t, m*Js + jh*2 + e].
    bf16 hw-pairs are bitcast to fp32 so this is a plain 4-byte 32x32
    block transpose, and both the input AP (jb, t) and output AP (jh, cl)
    are stride-1 in their innermost dim (full 1 elem/cycle/lane rate).
  - Softmax on the replicated [128, 32] energy; residual is FUSED into the
    attention weight: B = alpha*A + I, built as a block-diagonal [128,128]
    bf16 weight W (B^T per 32x32 diagonal block). One weight load serves
    all phase-2 matmuls; the full-partition contraction with block-diag W
    gives alpha*(A@q) + q per group. alpha=0 stays bitwise exact.
  - Phase-2 evacuation is a pure copy PSUM fp32 -> SBUF bf16, round-robin
    across scalar/vector/gpsimd; stores are 3.5KB-per-partition DMAs from
    a contiguous store buffer. y returns folded bf16; host de-folds and
    upcasts.

HBM traffic: 6.4MB in + 6.4MB out per core (vs 25.7MB for fp32).
"""

import sys

sys.path.insert(0, "/opt/trn_rl_repo")

from contextlib import ExitStack

import numpy as np

import concourse.bass as bass
import concourse.tile as tile
from concourse import bacc, mybir

# Problem shape (hardcoded per contract)
N, C, T, H, W = 8, 128, 32, 28, 28
HB = H * W  # 784
F = T * HB  # 25088
G = 4  # partition groups (c blocks of 32)
CL = 32  # c-local within group
NCORES = 8

f32 = mybir.dt.float32
bf16 = mybir.dt.bfloat16
AF = mybir.ActivationFunctionType
ALU = mybir.AluOpType
AX = mybir.AxisListType

NSLOT = 7  # chunks
Js = HB // NSLOT  # 112 hw per chunk
SW = T * Js  # 3584 chunk width (bf16 cols)
EP = 4  # hw packed per energy matmul group
JB = Js // EP  # 28 energy groups per chunk
NMM = 448  # phase-2 moving cols per matmul
KGRP = 1  # psum banks per phase-2 tile
NK = SW // NMM  # 8 phase-2 matmuls per chunk


def build_nc(
    nloads: int = 1,  # dma_starts per chunk load
    evac_engines: tuple = ("scalar", "vector"),  # gpsimd cannot read PSUM
    defer_last_t: bool = True,
):
    nc = bacc.Bacc(trn_type="TRN2", target_bir_lowering=False, debug=False)

    x = nc.declare_dram_parameter("x", [C, F], bf16, isOutput=False)
    al = nc.declare_dram_parameter("alpha_rep", [C, 1], f32, isOutput=False)
    sel4 = nc.declare_dram_parameter("sel4", [C, 4 * C], bf16, isOutput=False)
    id32 = nc.declare_dram_parameter("ident32", [C, T], f32, isOutput=False)
    y = nc.declare_dram_parameter("y", [C, F], bf16, isOutput=True)

    with ExitStack() as ctx:
        tc = ctx.enter_context(tile.TileContext(nc))
        consts = ctx.enter_context(tc.tile_pool(name="consts", bufs=1))
        smalls = ctx.enter_context(tc.tile_pool(name="smalls", bufs=1))
        xn_pool = ctx.enter_context(tc.tile_pool(name="xn", bufs=1))
        qt_pool = ctx.enter_context(tc.tile_pool(name="qt", bufs=1))
        psE_stack = ExitStack()
        psE = psE_stack.enter_context(tc.tile_pool(name="psE", bufs=1, space="PSUM"))

        W128 = smalls.tile([C, C], bf16)
        nc.gpsimd.memset(W128[:], 0.0)
        alpha_sb = consts.tile([C, 1], f32)
        sel_sb = consts.tile([C, 4 * C], bf16)
        id_sb = consts.tile([C, T], f32)
        warm = consts.tile([C, 1], f32)

        XN = xn_pool.tile([C, F], bf16)
        QT = qt_pool.tile([C, F], bf16)

        def emit_const_loads():
            # issued on the sync queue AFTER the x chunk issues (DMA issues
            # are async; consts are only needed from the softmax onwards)
            nc.sync.dma_start(alpha_sb[:], al[:])
            nc.sync.dma_start(sel_sb[:], sel4[:])
            nc.sync.dma_start(id_sb[:], id32[:])
            # Warm the Exp activation table (overlaps with phase-1 DMA).
            nc.scalar.activation(warm[:], alpha_sb[:], AF.Exp)

        def emit_transpose(m):
            # fp32-pair 32x32 block transpose: fold chunk m into QT.
            # in cells (jb, j4p, t) fp32; out cells (jh=2jb+j4p, cl) fp32.
            inf = (
                XN[:, m * SW : (m + 1) * SW]
                .bitcast(f32)
                .rearrange("p (jb j4p t) -> p jb j4p t", t=T, j4p=2)
            )
            outf = (
                QT[:, m * SW : (m + 1) * SW]
                .bitcast(f32)
                .rearrange("p (jb j4p cl) -> p jb j4p cl", cl=CL, j4p=2)
            )
            for j4p in range(2):
                nc.vector.transpose(outf[:, :, j4p, :], inf[:, :, j4p, :])

        # ---- Phase 1: load + energy + transpose-to-folded ----
        P4 = psE.tile([C, C], f32)
        LD = SW // nloads
        for m in range(NSLOT):
            for h in range(nloads):
                a0 = m * SW + h * LD
                nc.sync.dma_start(XN[:, a0 : a0 + LD], x[:, a0 : a0 + LD])
            if m == NSLOT - 1:
                emit_const_loads()
            for jb in range(JB):
                a = XN[:, m * SW + jb * (T * EP) : m * SW + (jb + 1) * (T * EP)]
                gidx = m * JB + jb
                nc.tensor.matmul(
                    P4[:],
                    a,
                    a,
                    start=(gidx == 0),
                    stop=(gidx == NSLOT * JB - 1),
                )
            if not (defer_last_t and m == NSLOT - 1):
                emit_transpose(m)

        # ---- Softmax -> W128 (block-diag B^T, B = alpha*A + I) ----
        P4sb = smalls.tile([C, C], bf16)
        nc.scalar.copy(P4sb[:], P4[:])
        Erep = psE.tile([C, T], f32)
        # P4 cols are (j4p, t, e); block (j4p, e) holds a stride-2 diagonal
        p4v = P4sb[:].rearrange("p (a t b) -> p a b t", a=2, b=2)
        for jj in range(EP):
            nc.tensor.matmul(
                Erep[:],
                sel_sb[:, jj * C : (jj + 1) * C],
                p4v[:, jj >> 1, jj & 1, :],
                start=(jj == 0),
                stop=(jj == EP - 1),
            )
        negmax = smalls.tile([C, 1], f32)
        nc.vector.tensor_reduce(
            negmax[:], Erep[:], axis=AX.X, op=ALU.max, negate=True
        )
        P = smalls.tile([C, T], f32)
        ssum = smalls.tile([C, 1], f32)
        nc.scalar.activation(
            P[:], Erep[:], AF.Exp, bias=negmax[:], scale=1.0, accum_out=ssum[:]
        )
        rcp = smalls.tile([C, 1], f32)
        nc.vector.reciprocal(rcp[:], ssum[:])
        Bp = smalls.tile([C, T], f32)
        nc.vector.tensor_scalar(
            out=Bp[:],
            in0=P[:],
            scalar1=rcp[:],
            scalar2=alpha_sb[:],
            op0=ALU.mult,
            op1=ALU.mult,
        )
        nc.vector.tensor_add(Bp[:], Bp[:], id_sb[:])
        Bt = smalls.tile([C, T], f32)
        nc.vector.transpose(Bt[:], Bp[:])
        for g in range(G):
            eng = nc.scalar if g % 2 == 0 else nc.gpsimd
            blk = (
                W128[g * CL : (g + 1) * CL, g * CL : (g + 1) * CL],
                Bt[g * CL : (g + 1) * CL, :],
            )
            if eng is nc.scalar:
                nc.scalar.copy(*blk)
            else:
                nc.gpsimd.tensor_copy(*blk)
        if defer_last_t:
            emit_transpose(NSLOT - 1)
        psE_stack.close()  # release P4/Erep PSUM banks for phase 2

        # ---- Phase 2: fused attention+residual matmul + store ----
        n_evac = 0
        with ExitStack() as p2:
            ps2 = p2.enter_context(tc.tile_pool(name="ps2", bufs=8, space="PSUM"))
            ysb_pool = p2.enter_context(tc.tile_pool(name="ysb", bufs=2))
            for m in range(NSLOT):
                ysb = ysb_pool.tile([C, SW], bf16, tag="ysb")
                for kb in range(NK // KGRP):
                    ps = ps2.tile([C, KGRP * 512], f32)
                    for b in range(KGRP):
                        col0 = m * SW + (kb * KGRP + b) * NMM
                        nc.tensor.matmul(
                            ps[:, b * 512 : b * 512 + NMM],
                            W128[:],
                            QT[:, col0 : col0 + NMM],
                            start=True,
                            stop=True,
                        )
                    eng = {
                        "scalar": nc.scalar,
                        "vector": nc.vector,
                        "gpsimd": nc.gpsimd,
                    }[evac_engines[n_evac % len(evac_engines)]]
                    n_evac += 1
                    a0 = kb * KGRP * NMM
                    dst = ysb[:, a0 : a0 + KGRP * NMM].rearrange(
                        "p (b j) -> p b j", b=KGRP
                    )
                    src = ps[:].rearrange("p (b r) -> p b r", b=KGRP)[:, :, 0:NMM]
                    if eng is nc.scalar:
                        nc.scalar.copy(dst, src)
                    else:
                        eng.tensor_copy(dst, src)
                    # store per 4 evac tiles (1792 cols -> 3.5KB packets)
                    if kb % 4 == 3:
                        s0 = (kb - 3) * KGRP * NMM
                        nc.sync.dma_start(
                            y[:, m * SW + s0 : m * SW + s0 + 4 * KGRP * NMM],
                            ysb[:, s0 : s0 + 4 * KGRP * NMM],
                        )

    nc.compile()
    return nc


def _consts():
    # P4 rows are (j4p, t, e); selector block jj=(j4p, e) extracts that
    # stride-2 diagonal and replicates it to all 4 partition groups:
    # sel[64*j4p + 2*t + e, (2*j4p+e)*C + 32*g + t] = 1
    sel = np.zeros((C, 4 * C), np.float32)
    for j4p in range(2):
        for e in range(2):
            jj = 2 * j4p + e
            for t in range(T):
                for g in range(G):
                    sel[64 * j4p + 2 * t + e, jj * C + g * 32 + t] = 1.0
    id32 = np.zeros((C, T), np.float32)
    for p in range(C):
        id32[p, p % T] = 1.0
    return sel, id32


_BUILD_KW = dict()


def make_in_maps(x: np.ndarray, alpha: np.ndarray):
    import ml_dtypes

    assert x.shape == (N, C, T, H, W) and x.dtype == np.float32
    sel, id32 = _consts()
    sel_bf = sel.astype(ml_dtypes.bfloat16)
    alpha_rep = np.full((C, 1), np.float32(alpha.reshape(-1)[0]), np.float32)
    # energy cell layout: xc[c, m, jb, j4p, t, e] = x[c, t, m*Js+jb*4+j4p*2+e]
    xr = (
        x.reshape(N, C, T, NSLOT, JB, 2, 2)
        .transpose(0, 1, 3, 4, 5, 2, 6)
        .reshape(N, C, F)
        .astype(ml_dtypes.bfloat16)
    )
    xr = np.ascontiguousarray(xr)
    return [
        {"x": xr[n], "alpha_rep": alpha_rep, "sel4": sel_bf, "ident32": id32}
        for n in range(NCORES)
    ]


def unfold_y(yf: np.ndarray) -> np.ndarray:
    # yf[32g+t, m*SW + jh*64 + cl*2 + e] = out[32g+cl, t, m*Js + jh*2 + e]
    return (
        np.asarray(yf)
        .astype(np.float32)
        .reshape(G, T, NSLOT, Js // 2, CL, 2)
        .transpose(0, 4, 1, 2, 3, 5)
        .reshape(C, T, H, W)
    )


def kernel(x: np.ndarray, alpha: np.ndarray) -> np.ndarray:
    from concourse.bass_utils import run_bass_kernel_spmd

    nc = build_nc(**_BUILD_KW)
    in_maps = make_in_maps(x, alpha)
    res = run_bass_kernel_spmd(nc, in_maps, list(range(NCORES)))
    out = np.stack([unfold_y(res.results[n]["y"]) for n in range(NCORES)])
    return out.astype(np.float32)
